# revision 1
# baseline (speedup 1.0000x reference)
"""Trainium2 Bass kernel for nn_BaselineAttention_36172214567310.

Reference computation (note the einsum 'bhqk,bhkd->bhkd' bug: the "attention
output" is v scaled by the column-sums of the softmax matrix):

    qkv = x @ w_qkv                       # [b, s, 3*H*D]
    q, k, v = split(qkv)                  # per head
    P = softmax(q @ k^T / sqrt(D))        # [q, k] rows sum to 1
    colsum[k] = sum_q P[q, k]
    values[k, :] = v[k, :] * colsum_h[k]
    out = values @ w_o

Sharding: 8 cores = 2 batches x 4 head-groups (4 heads each). Each core
computes qkv^T for its heads, scores + exp (fused rowsum on ACT) + colsum
matvec per head, scales v^T, and a partial out = values @ w_o_slice.
Host sums the 4 partials per batch.

All matmuls run in float32r (fp32 rounded to 11-bit mantissa; full PE rate).
"""

import sys

sys.path.insert(0, "/opt/trn_rl_repo")

import numpy as np

B, S, HIDDEN = 2, 2048, 1024
NH, HD = 16, 64
HPC = 4            # heads per core
FPC = 3 * HPC * HD # qkv feature columns per core (768)
N_CORES = 8
P = 128
NT = S // 512      # 512-column tiles over the sequence (4)
QC = S // P        # 128-row q chunks (16)


def round_f32r(a: np.ndarray) -> np.ndarray:
    """Round fp32 to the fp32r grid (11-bit mantissa, round-to-nearest-even)."""
    u = np.ascontiguousarray(a).view(np.uint32)
    low = u & np.uint32(0x00000FFF)
    base = u & np.uint32(0xFFFFF000)
    lsb = (u >> np.uint32(12)) & np.uint32(1)
    round_up = (low > 0x800) | ((low == 0x800) & (lsb == 1))
    out = base + (round_up.astype(np.uint32) << np.uint32(12))
    return out.view(np.float32)


_CACHE = {}


def _build():
    if "nc" in _CACHE:
        return _CACHE["nc"]

    import concourse.bass as bass
    import concourse.mybir as mybir
    import concourse.tile as tile
    from concourse import bacc
    from concourse.tile_rust import add_dep_helper

    F32 = mybir.dt.float32
    F32R = mybir.dt.float32r
    BF16 = mybir.dt.bfloat16
    EXP = mybir.ActivationFunctionType.Exp

    nc = bacc.Bacc()
    xT_d = nc.declare_dram_parameter("xT", [HIDDEN, S], F32R, isOutput=False)
    xTb_d = nc.declare_dram_parameter("xTb", [HIDDEN, S], BF16, isOutput=False)
    wq_d = nc.declare_dram_parameter("wq", [HIDDEN, HPC * HD], F32R, isOutput=False)
    wqb_d = nc.declare_dram_parameter("wqb", [HIDDEN, 2 * HPC * HD], BF16,
                                      isOutput=False)
    wo_d = nc.declare_dram_parameter("wo", [HPC * HD, HIDDEN], F32R, isOutput=False)
    out_d = nc.declare_dram_parameter("out", [S, HIDDEN], F32, isOutput=True)

    with tile.TileContext(nc) as tc:
        # persistent pools
        with tc.tile_pool(name="qkvt", bufs=1) as qkvt_pool, \
             tc.tile_pool(name="wq", bufs=1) as wq_pool, \
             tc.tile_pool(name="wo", bufs=1) as wo_pool:

            # ---- phase 1: qkv^T = (x @ w_qkv)^T for this core's heads ----
            # qkvT tiles: mc 0,1 = Q (2 heads each), 2,3 = K, 4,5 = V
            qkvt = [qkvt_pool.tile([P, S], BF16 if mc < 4 else F32R,
                                   name=f"qkvt{mc}") for mc in range(6)]
            wq_t = [wq_pool.tile([P, HPC * HD], F32R, name=f"wq{kc}")
                    for kc in range(8)]
            wqb_t = [wq_pool.tile([P, 2 * HPC * HD], BF16, name=f"wqb{kc}")
                     for kc in range(8)]
            for kc in range(8):
                nc.sync.dma_start(out=wq_t[kc], in_=wq_d[kc * P:(kc + 1) * P, :])
                nc.sync.dma_start(out=wqb_t[kc], in_=wqb_d[kc * P:(kc + 1) * P, :])
            wo_t = [wo_pool.tile([P, HIDDEN], F32R, name=f"wo{kc}") for kc in range(2)]
            for kc in range(2):
                nc.sync.dma_start(out=wo_t[kc], in_=wo_d[kc * P:(kc + 1) * P, :])

            with tc.tile_pool(name="xt", bufs=1) as xt_pool, \
                 tc.tile_pool(name="ps_qkv", bufs=4, space="PSUM") as ps_qkv:
                xtb = [xt_pool.tile([P, S], BF16, name=f"xtb{kc}") for kc in range(8)]
                for kc in range(8):
                    nc.sync.dma_start(out=xtb[kc], in_=xTb_d[kc * P:(kc + 1) * P, :])
                xt = [xt_pool.tile([P, S], F32R, name=f"xt{kc}") for kc in range(8)]
                for kc in range(8):
                    nc.sync.dma_start(out=xt[kc], in_=xT_d[kc * P:(kc + 1) * P, :])
                # Q, K projections in bf16 (mc 0-3), V in f32r (mc 4, 5)
                for mc in range(6):
                    for nt in range(NT):
                        ps = ps_qkv.tile([P, 512], F32, name="psq")
                        for kc in range(8):
                            if mc < 4:
                                nc.tensor.matmul(
                                    ps, wqb_t[kc][:, mc * P:(mc + 1) * P],
                                    xtb[kc][:, nt * 512:(nt + 1) * 512],
                                    start=(kc == 0), stop=(kc == 7))
                            else:
                                nc.tensor.matmul(
                                    ps, wq_t[kc][:, (mc - 4) * P:(mc - 3) * P],
                                    xt[kc][:, nt * 512:(nt + 1) * 512],
                                    start=(kc == 0), stop=(kc == 7))
                        nc.vector.tensor_copy(
                            out=qkvt[mc][:, nt * 512:(nt + 1) * 512], in_=ps)

            # ---- phase 2+3: per-head colsums (replicated across 64
            # partitions via a replicated matvec lhsT), then v^T *= colsum ----
            H = S // 2  # 1024-col half chunks so scores double-buffer in PSUM
            with tc.tile_pool(name="esb", bufs=6) as e_pool, \
                 tc.tile_pool(name="rs", bufs=8) as rs_pool, \
                 tc.tile_pool(name="ps_s", bufs=2, space="PSUM") as ps_s_pool, \
                 tc.tile_pool(name="ps_c", bufs=1, space="PSUM") as ps_c_pool:
                for j in range(HPC):
                    # colsum for head j, replicated across all 128 partitions
                    # (f32r matmuls require dst base partition 0)
                    qt = qkvt[j // 2]
                    kt = qkvt[2 + j // 2]
                    vt = qkvt[4 + j // 2]
                    bp = (j % 2) * 64
                    ps_c = ps_c_pool.tile([P, S], F32, name="psc")

                    def emit_matvec(pend):
                        wr_p, e_p, qc_p = pend
                        for hh in range(2):
                            for nt in range(2):
                                nc.tensor.matmul(
                                    ps_c[:, hh * H + nt * 512:
                                         hh * H + (nt + 1) * 512],
                                    wr_p,
                                    e_p[hh][:, nt * 512:(nt + 1) * 512],
                                    start=(qc_p == 0), stop=(qc_p == QC - 1))

                    pending = None  # software-pipelined matvec of qc-1
                    for qc in range(QC):
                        e_sb = []
                        rs_h = []
                        for hh in range(2):
                            ps_s = ps_s_pool.tile([P, H], F32, name="pss")
                            for nt in range(2):
                                nc.tensor.matmul(
                                    ps_s[:, nt * 512:(nt + 1) * 512],
                                    qt[bp:bp + 64, qc * P:(qc + 1) * P],
                                    kt[bp:bp + 64,
                                       hh * H + nt * 512:hh * H + (nt + 1) * 512],
                                    start=True, stop=True)
                            e = e_pool.tile([P, H], BF16, name="esb")
                            r = rs_pool.tile([P, 1], F32, name=f"rs{hh}")
                            # E = exp(scores / 8), rowsum fused on ACT
                            nc.scalar.activation(e, ps_s, EXP, scale=0.125,
                                                 accum_out=r)
                            e_sb.append(e)
                            rs_h.append(r)
                        # emit the PREVIOUS chunk's matvec here so PE can run
                        # it while ACT works on this chunk's exps
                        if pending is not None:
                            emit_matvec(pending)
                        rs = rs_pool.tile([P, 1], F32, name="rs")
                        nc.vector.tensor_tensor(rs, rs_h[0], rs_h[1],
                                                mybir.AluOpType.add)
                        wr = rs_pool.tile([P, P], BF16, name="wr")
                        nc.vector.reciprocal(rs, rs)
                        nc.vector.tensor_copy(
                            out=wr, in_=rs.to_broadcast([P, P]))
                        pending = (wr, e_sb, qc)
                    emit_matvec(pending)
                    # v^T *= colsum for this head's 64 partitions
                    nc.vector.tensor_tensor(
                        vt[bp:bp + 64, :], vt[bp:bp + 64, :],
                        ps_c[bp:bp + 64, :], mybir.AluOpType.mult)

            # ---- phase 4: out_partial = values @ w_o_slice  [s, hidden] ----
            with tc.tile_pool(name="osb", bufs=3) as o_pool, \
                 tc.tile_pool(name="ps_o", bufs=2, space="PSUM") as ps_o_pool:
                for sc in range(QC):
                    ps_o = ps_o_pool.tile([P, HIDDEN], F32, name="pso")
                    for nh in range(2):
                        for kc in range(2):
                            nc.tensor.matmul(
                                ps_o[:, nh * 512:(nh + 1) * 512],
                                qkvt[4 + kc][:, sc * P:(sc + 1) * P],
                                wo_t[kc][:, nh * 512:(nh + 1) * 512],
                                start=(kc == 0), stop=(kc == 1))
                    o_sb = o_pool.tile([P, HIDDEN], F32, name="osb")
                    nc.vector.tensor_copy(out=o_sb, in_=ps_o)
                    nc.sync.dma_start(out=out_d[sc * P:(sc + 1) * P, :], in_=o_sb)

    nc.compile()
    _CACHE["nc"] = nc
    return nc


def kernel(x: np.ndarray, w_qkv: np.ndarray, w_o: np.ndarray) -> np.ndarray:
    import ml_dtypes
    from concourse.bass_utils import run_bass_kernel_spmd

    nc = _build()

    xT = [round_f32r(np.ascontiguousarray(x[b].T)) for b in range(B)]
    xTb = [np.ascontiguousarray(x[b].T).astype(ml_dtypes.bfloat16) for b in range(B)]
    in_maps = []
    for c in range(N_CORES):
        b, g = divmod(c, HPC)
        wqk_slice = np.concatenate(
            [w_qkv[:, t * NH * HD + 256 * g: t * NH * HD + 256 * g + 256]
             for t in range(2)], axis=1)
        wv_slice = w_qkv[:, 2 * NH * HD + 256 * g: 2 * NH * HD + 256 * g + 256]
        wo_slice = w_o[256 * g:256 * g + 256, :]
        in_maps.append({
            "xT": xT[b],
            "xTb": xTb[b],
            "wq": round_f32r(wv_slice),
            "wqb": wqk_slice.astype(ml_dtypes.bfloat16),
            "wo": round_f32r(wo_slice),
        })

    res = run_bass_kernel_spmd(nc, in_maps, list(range(N_CORES)),
                               **_CACHE.get("run_kwargs", {}))
    _CACHE["last_result"] = res

    out = np.zeros((B, S, HIDDEN), np.float32)
    for c in range(N_CORES):
        out[c // HPC] += res.results[c]["out"]
    return out



# revision 2
# speedup vs baseline: 1.0108x; 1.0108x over previous
"""Trainium2 Bass kernel for nn_BaselineAttention_36172214567310 (v4).

Reference computation (einsum 'bhqk,bhkd->bhkd' sums over q, so attention
collapses to: v scaled by softmax column-sums):

    qkv = x @ w_qkv
    P = softmax(q @ k^T / 8)      per head, rows sum to 1
    colsum[k] = sum_q P[q, k]
    out = (v * colsum[:, None]) @ w_o

Sharding: 8 cores = 2 batches x 4 head-groups (4 heads each).

v7 = v6 - LDW padding (measured: hurts) + bf16 outputs (the three
partial outputs totalled 24MB fp32 of DMA writes and made the tail
DMA-bound; bf16 halves it, host sums in fp32).
v5 = v4 + q/k projections in fp8 DoubleRow (half the MMs, shorter lead).
v4: uniform PE instruction density to keep the HAM clock-gate warm:
- warmup MMs at t=0 (no DMA dependency) so the lead projection runs at
  2.4 GHz; lead is only q01-h0 + k01.
- one global filler queue (rest of the qkv projection, then the output
  projection in four readiness-gated quarter phases) paced at ~3.6
  instructions per chunk under the scores+exp+matvec steady loop.
- output projection is split into three DRAM outputs (v01 x wo0 k-half,
  v23-head2 rows, v23-head3 rows) summed on the host, so nearly all of
  P4 streams out mid-kernel.
- lagged DoubleRow fp8 matvec (no bursts): head j k-half1 runs during
  head j+1 chunks 0-7, k-half0 during own chunks 8-15.
"""

import sys

sys.path.insert(0, "/opt/trn_rl_repo")

import numpy as np

B, S, HIDDEN = 2, 2048, 1024
NH, HD = 16, 64
HPC = 4
N_CORES = 8
P = 128
QC = S // P
NPAIR = QC // 2

_CACHE = {}


def _build():
    if "nc" in _CACHE:
        return _CACHE["nc"]

    import concourse.mybir as mybir
    import concourse.tile as tile
    from concourse import bacc

    F32 = mybir.dt.float32
    BF16 = mybir.dt.bfloat16
    FP8 = mybir.dt.float8e4
    EXP = mybir.ActivationFunctionType.Exp
    COPY = mybir.ActivationFunctionType.Copy
    ADD = mybir.AluOpType.add
    MULT = mybir.AluOpType.mult
    DR = mybir.MatmulPerfMode.DoubleRow

    nc = bacc.Bacc()
    xT_d = nc.declare_dram_parameter("xT", [HIDDEN, S], BF16, isOutput=False)
    x8_d = nc.declare_dram_parameter("x8", [HIDDEN // 2, 2 * S], FP8, isOutput=False)
    w8_d = nc.declare_dram_parameter("w8", [HIDDEN // 2, 2 * 512], FP8, isOutput=False)
    wqkv_d = nc.declare_dram_parameter("wqkv", [HIDDEN, 256], BF16, isOutput=False)
    wo_d = nc.declare_dram_parameter("wo", [256, HIDDEN], BF16, isOutput=False)
    out_d = nc.declare_dram_parameter("out", [S, HIDDEN], BF16, isOutput=True)
    out2_d = nc.declare_dram_parameter("out2", [S, HIDDEN], BF16, isOutput=True)
    out3_d = nc.declare_dram_parameter("out3", [S, HIDDEN], BF16, isOutput=True)

    with tile.TileContext(nc) as tc:
        with tc.tile_pool(name="persist", bufs=1) as sb, \
             tc.tile_pool(name="small", bufs=1) as sm, \
             tc.tile_pool(name="rsp", bufs=8) as rsp, \
             tc.tile_pool(name="outp", bufs=3) as outp, \
             tc.tile_pool(name="ps_s", bufs=2, space="PSUM") as ps_s_pool, \
             tc.tile_pool(name="ps_c", bufs=1, space="PSUM") as ps_c_pool, \
             tc.tile_pool(name="ps_f", bufs=1, space="PSUM") as ps_f_pool:

            xt = [sb.tile([P, S], BF16, name=f"xt{kc}") for kc in range(8)]
            x8t = [sb.tile([P, 2, S], FP8, name=f"x8t{p}") for p in range(4)]
            w8t = [sb.tile([P, 2, 512], FP8, name=f"w8t{p}") for p in range(4)]
            wq_t = [sb.tile([P, 256], BF16, name=f"wq{kc}") for kc in range(8)]
            wo_t = [sb.tile([P, HIDDEN], BF16, name=f"wo{kc}") for kc in range(2)]
            qkvt = [sb.tile([P, S], BF16, name=f"qkvt{mc}") for mc in range(6)]
            e8 = [sb.tile([P, QC, S], FP8, name=f"e8_{i}") for i in range(2)]
            wr8 = [sb.tile([P, QC, P], FP8, name=f"wr8_{i}") for i in range(2)]
            dum = sm.tile([P, 1], F32, name="dum")
            dum2 = sm.tile([P, 1], F32, name="dum2")
            wsrc = sm.tile([P, 512], BF16, name="wsrc")

            # exp table preload + PE warmup (no DMA dependency)
            nc.vector.memset(dum, 0.0)
            nc.scalar.activation(dum2, dum, EXP)
            nc.vector.memset(wsrc, 0.0)
            wps = ps_c_pool.tile([P, 1024], F32, name="psc")
            for i in range(20):
                nc.tensor.matmul(wps[:, 0:512], wsrc[:, 0:128], wsrc,
                                 start=True, stop=True)

            for p in range(4):
                nc.sync.dma_start(out=w8t[p], in_=w8_d[p * P:(p + 1) * P, :])
                nc.sync.dma_start(out=x8t[p], in_=x8_d[p * P:(p + 1) * P, :])
            for kc in range(8):
                nc.sync.dma_start(out=wq_t[kc],
                                  in_=wqkv_d[kc * P:(kc + 1) * P, :])
                nc.sync.dma_start(out=xt[kc], in_=xT_d[kc * P:(kc + 1) * P, :])
            for kc in range(2):
                nc.sync.dma_start(out=wo_t[kc],
                                  in_=wo_d[kc * P:(kc + 1) * P, :])

            # ---------------- global filler queue ----------------
            # p1 item: ("p1", ready, mc, hh, kc, n) - one 512-col MM of the
            #   qkv projection (16 MMs per (mc,hh) accumulation + copy).
            # p4 item: ("p4", ready, which, sc, n) - output projection MM.
            queue = []
            for dk in range(4):                      # q01-h1 (DR), deadline gc 8
                for n in range(2):
                    queue.append(("qk", 0, 0, 1, dk, n))
            for mc in (2,):                          # v01 (bf16)
                for hh in range(2):
                    for kc in range(8):
                        for n in range(2):
                            queue.append(("p1", 0, mc, hh, kc, n))
            for mc in (1, 2):                        # q23, k23 (DR; m-block 2,3... mapped below)
                for hh in range(2):
                    for dk in range(4):
                        for n in range(2):
                            queue.append(("qk", 0, mc + 1, hh, dk, n))
            for mc in (5,):                          # v23 (bf16)
                for hh in range(2):
                    for kc in range(8):
                        for n in range(2):
                            queue.append(("p1", 0, mc, hh, kc, n))
            # p4 quarter phases:
            # A: out2 = v01^T x wo0            (full K=128), sc 0-7 ready 33,
            #    sc 8-15 ready 41
            # B: out3 = v23[head2 rows] x wo1  (K=64),      sc 0-7 ready 49,
            #    sc 8-15 ready 57
            for sc in range(8):
                for n in range(2):
                    queue.append(("p4", 33, "A", sc, n))
            for sc in range(8, QC):
                for n in range(2):
                    queue.append(("p4", 41, "A", sc, n))
            for sc in range(8):
                for n in range(2):
                    queue.append(("p4", 49, "B", sc, n))
            for sc in range(8, QC):
                for n in range(2):
                    queue.append(("p4", 57, "B", sc, n))

            f_state = {"i": 0, "ps": None, "ps4": None}

            def emit_item(it):
                if it[0] == "qk":
                    _, _, mb, hh, dk, n = it
                    if dk == 0 and n == 0:
                        f_state["ps"] = ps_f_pool.tile([P, 1024], F32,
                                                       name="psf")
                    ps = f_state["ps"]
                    c0 = hh * 1024 + n * 512
                    nc.tensor.matmul(
                        ps[:, n * 512:(n + 1) * 512],
                        w8t[dk][:, :, mb * P:(mb + 1) * P],
                        x8t[dk][:, :, c0:c0 + 512],
                        perf_mode=DR,
                        start=(dk == 0), stop=(dk == 3))
                    if dk == 3 and n == 1:
                        qdst = {0: 0, 2: 3, 3: 4}[mb]
                        nc.vector.tensor_copy(
                            out=qkvt[qdst][:, hh * 1024:(hh + 1) * 1024],
                            in_=ps)
                    return
                if it[0] == "p1":
                    _, _, mc, hh, kc, n = it
                    if kc == 0 and n == 0:
                        f_state["ps"] = ps_f_pool.tile([P, 1024], F32,
                                                       name="psf")
                    ps = f_state["ps"]
                    c0 = hh * 1024 + n * 512
                    wcol = 0 if mc == 2 else P
                    nc.tensor.matmul(
                        ps[:, n * 512:(n + 1) * 512],
                        wq_t[kc][:, wcol:wcol + P],
                        xt[kc][:, c0:c0 + 512],
                        start=(kc == 0), stop=(kc == 7))
                    if kc == 7 and n == 1:
                        nc.vector.tensor_copy(
                            out=qkvt[mc][:, hh * 1024:(hh + 1) * 1024], in_=ps)
                else:
                    _, _, which, sc, n = it
                    if n == 0:
                        f_state["ps4"] = ps_f_pool.tile([P, 1024], F32,
                                                        name="psf")
                    ps4 = f_state["ps4"]
                    if which == "A":
                        lhsT = qkvt[2][:, sc * P:(sc + 1) * P]
                        rhs = wo_t[0][:, n * 512:(n + 1) * 512]
                        dst = out2_d
                    else:
                        lhsT = qkvt[5][0:64, sc * P:(sc + 1) * P]
                        rhs = wo_t[1][0:64, n * 512:(n + 1) * 512]
                        dst = out3_d
                    nc.tensor.matmul(ps4[:, n * 512:(n + 1) * 512], lhsT, rhs,
                                     start=True, stop=True)
                    if n == 1:
                        o_sb = outp.tile([P, HIDDEN], BF16, name="osb")
                        nc.vector.tensor_copy(out=o_sb, in_=ps4)
                        nc.sync.dma_start(out=dst[sc * P:(sc + 1) * P, :],
                                          in_=o_sb)

            def pump(gc, budget):
                while budget > 0 and f_state["i"] < len(queue):
                    it = queue[f_state["i"]]
                    if it[1] > gc:
                        return
                    f_state["i"] += 1
                    emit_item(it)
                    budget -= 1

            def target(gc):
                if gc < 8:
                    return int(4.5 * (gc + 1))
                if gc <= 46:
                    return 36 + int(2.75 * (gc - 7))
                return min(len(queue), 137 + 2 * (gc - 46))

            # ---------------- lead: q01-h0, k01 ----------------
            def emit_lead(mb, qdst, hh):
                ps = ps_s_pool.tile([P, 1024], F32, name="pss")
                for dk in range(4):
                    for n in range(2):
                        c0 = hh * 1024 + n * 512
                        nc.tensor.matmul(
                            ps[:, n * 512:(n + 1) * 512],
                            w8t[dk][:, :, mb * P:(mb + 1) * P],
                            x8t[dk][:, :, c0:c0 + 512],
                            perf_mode=DR,
                            start=(dk == 0), stop=(dk == 3))
                nc.scalar.activation(
                    qkvt[qdst][:, hh * 1024:(hh + 1) * 1024], ps, COPY)

            emit_lead(0, 0, 0)
            emit_lead(1, 1, 0)
            emit_lead(1, 1, 1)

            # ---------------- head loop ----------------
            def matvec(j, half, pr, first, last):
                eb, wb = e8[j % 2], wr8[j % 2]
                psc = f_state["psc"]
                for n in range(2):
                    c0 = half * 1024 + n * 512
                    nc.tensor.matmul(
                        psc[:, n * 512:(n + 1) * 512],
                        wb[:, 2 * pr:2 * pr + 2, :],
                        eb[:, 2 * pr:2 * pr + 2, c0:c0 + 512],
                        perf_mode=DR,
                        start=first, stop=last)

            def vscale(j, half):
                vt = qkvt[2 if j < 2 else 5]
                bp = (j % 2) * 64
                psc = f_state["psc"]
                c0 = half * 1024
                nc.vector.tensor_tensor(
                    vt[bp:bp + 64, c0:c0 + 1024], vt[bp:bp + 64, c0:c0 + 1024],
                    psc[bp:bp + 64, :], MULT)

            for j in range(HPC):
                qt = qkvt[0 if j < 2 else 3]
                kt = qkvt[1 if j < 2 else 4]
                bp = (j % 2) * 64
                eb, wb = e8[j % 2], wr8[j % 2]

                for qc in range(QC):
                    gc = j * QC + qc
                    r_h = [None, None]
                    for hh in range(2):
                        ps_s = ps_s_pool.tile([P, 1024], F32, name="pss")
                        for n in range(2):
                            c0 = hh * 1024 + n * 512
                            nc.tensor.matmul(
                                ps_s[:, n * 512:(n + 1) * 512],
                                qt[bp:bp + 64, qc * P:(qc + 1) * P],
                                kt[bp:bp + 64, c0:c0 + 512],
                                start=True, stop=True)
                        r = rsp.tile([P, 1], F32, name=f"r{hh}")
                        nc.scalar.activation(
                            eb[:, qc, hh * 1024:(hh + 1) * 1024],
                            ps_s, EXP, scale=0.125, accum_out=r)
                        r_h[hh] = r
                        if hh == 0:
                            due = target(gc) - f_state["i"]
                            pump(gc, max(0, min(3, (due + 1) // 2)))
                    rs = rsp.tile([P, 1], F32, name="rs")
                    nc.vector.tensor_tensor(rs, r_h[0], r_h[1], ADD)
                    rinv = rsp.tile([P, 1], F32, name="rinv")
                    nc.vector.reciprocal(rinv, rs)
                    nc.vector.tensor_scalar(wb[:, qc, :],
                                            rinv.to_broadcast([P, P]),
                                            1024.0, None, MULT)
                    has_mv = (qc < NPAIR and j > 0) or qc >= NPAIR
                    if qc < NPAIR and j > 0:
                        if qc == 0:
                            f_state["psc"] = ps_c_pool.tile(
                                [P, 1024], F32, name="psc")
                        matvec(j - 1, 1, qc, qc == 0, qc == NPAIR - 1)
                        if qc == NPAIR - 1:
                            vscale(j - 1, 1)
                    elif qc >= NPAIR:
                        pr = qc - NPAIR
                        if pr == 0:
                            f_state["psc"] = ps_c_pool.tile(
                                [P, 1024], F32, name="psc")
                        matvec(j, 0, pr, pr == 0, pr == NPAIR - 1)
                        if pr == NPAIR - 1:
                            vscale(j, 0)
                    pump(gc, max(0, min(5, target(gc) - f_state["i"])))

            # ---------------- tail ----------------
            # head 3 k-half1 matvec interleaved with out += v23[head3] x wo1
            # for s-chunks 0-7 (those only need the k-half0 v-scale, done).
            def p4bb(sc):
                ps_o = ps_s_pool.tile([P, 1024], F32, name="pss")
                for n in range(2):
                    nc.tensor.matmul(
                        ps_o[:, n * 512:(n + 1) * 512],
                        qkvt[5][64:128, sc * P:(sc + 1) * P],
                        wo_t[1][64:128, n * 512:(n + 1) * 512],
                        start=True, stop=True)
                o_sb = outp.tile([P, HIDDEN], BF16, name="osb")
                if sc % 2 == 0:
                    nc.scalar.activation(o_sb, ps_o, COPY)
                else:
                    nc.vector.tensor_copy(out=o_sb, in_=ps_o)
                nc.sync.dma_start(out=out_d[sc * P:(sc + 1) * P, :], in_=o_sb)

            f_state["psc"] = ps_c_pool.tile([P, 1024], F32, name="psc")
            for pr in range(NPAIR):
                matvec(3, 1, pr, pr == 0, pr == NPAIR - 1)
                pump(63, 2)
                p4bb(pr)
            vscale(3, 1)
            pump(63, len(queue))
            for sc in range(NPAIR, QC):
                p4bb(sc)

    nc.compile()
    _CACHE["nc"] = nc
    return nc


def kernel(x: np.ndarray, w_qkv: np.ndarray, w_o: np.ndarray) -> np.ndarray:
    import ml_dtypes
    from concourse.bass_utils import run_bass_kernel_spmd

    nc = _build()

    def pair_interleave(a):
        # [1024, C] -> [512, 2C]: rows 256p+128i+part -> row 128p+part,
        # col block i
        cc = a.shape[1]
        return np.ascontiguousarray(
            a.reshape(4, 2, 128, cc).transpose(0, 2, 1, 3).reshape(512, 2 * cc))

    def to_fp8(a):
        return np.clip(a, -240.0, 240.0).astype(ml_dtypes.float8_e4m3)

    xT = [np.ascontiguousarray(x[b].T).astype(ml_dtypes.bfloat16)
          for b in range(B)]
    x8 = [to_fp8(pair_interleave(np.ascontiguousarray(x[b].T)))
          for b in range(B)]
    in_maps = []
    for c in range(N_CORES):
        b, g = divmod(c, HPC)
        base = 256 * g
        q01 = w_qkv[:, base:base + 128]
        q23 = w_qkv[:, base + 128:base + 256]
        k01 = w_qkv[:, 1024 + base:1024 + base + 128]
        k23 = w_qkv[:, 1024 + base + 128:1024 + base + 256]
        v01 = w_qkv[:, 2048 + base:2048 + base + 128]
        v23 = w_qkv[:, 2048 + base + 128:2048 + base + 256]
        wqk = np.concatenate([q01, k01, q23, k23], axis=1)
        wv = np.concatenate([v01, v23], axis=1)
        wo_slice = w_o[base:base + 256, :] * (1.0 / 1024.0)
        in_maps.append({
            "xT": xT[b],
            "x8": x8[b],
            "w8": to_fp8(pair_interleave(wqk)),
            "wqkv": wv.astype(ml_dtypes.bfloat16),
            "wo": wo_slice.astype(ml_dtypes.bfloat16),
        })

    res = run_bass_kernel_spmd(nc, in_maps, list(range(N_CORES)),
                               **_CACHE.get("run_kwargs", {}))
    _CACHE["last_result"] = res

    out = np.zeros((B, S, HIDDEN), np.float32)
    for c in range(N_CORES):
        r = res.results[c]
        out[c // HPC] += (r["out"].astype(np.float32)
                          + r["out2"].astype(np.float32)
                          + r["out3"].astype(np.float32))
    return out
